# revision 2
# baseline (speedup 1.0000x reference)
"""Trainium2 Bass kernel for the C51-style categorical projection loss.

Algorithm (nibble-plane edition)
--------------------------------
Per batch row i (direction d_i in {0,1}, scalar skewness s):

    loss = -mean_i( w_i * (anchor_i @ P_{d_i}) . log(feature_i + 1e-16) )

P_d is a 51x51 projection depending only on +-s, so the loss reduces to two
51x51 contraction matrices  M_g[j,u] = sum_{i in g} (w a)[i,j] * lnf[i,u].

Device-side trick: fp8e4m3 bytes are affine-log.  For byte b = 16*hi + lo
(nibbles hi, lo in [0,15]), bitcasting a nibble byte as fp8e4m3 gives the
EXACT value nib * 2^-9 (subnormals + first normal octave are equi-spaced).
So with  lnf ~= C1*b + c0 + r(b):

    sum_i X[i,j] * lnf[i,u]
      ~= C1 * 2^9 * (16 * [X^T @ HI] + [X^T @ LO]) + (colsum X)[j] * K[u]

HI/LO are nibble planes split from the raw feature bytes with two u16
bitwise DVE ops (4x mode), and the contraction runs on the TensorEngine in
fp8 DoubleRow perf mode (256 batch rows per ldweights+matmul pair, both
planes streamed as one 208-column moving block).  K[u] (mean affine
residual per column) comes from host-side byte histograms - integer
bookkeeping only, no host transcendentals.

The anchor side ships as 4-bit: nibble = stochastic_round(w*|a| * 15),
two values per byte; the same DVE nibble-split yields exact fp8 weights
(value nib*2^-9, host rescales by 2^9/15).  Rows are pre-sorted by
direction (stable) so each core sees group-0 rows then group-1 rows,
each zero-padded to a fixed 130 k-tiles - no signs, no scatter; the
P_0/P_1 split is by PSUM region.

Schedules: feature DMA in 6 chunks/group for pipelining; anchor DMA+planes
in 3 coarse chunks/group (DVE instruction overhead dominates small ops).
DMAs alternate between the SP and Activation HWDGE queues to halve the
serialized per-queue stream time.

Sharding: pure data parallel over (sorted) batch rows, 8 cores.
"""

import os
import numpy as np
from contextlib import ExitStack

ATOMS = 51
V_MAX = 10.0
V_MIN = -10.0
DELTA = (V_MAX - V_MIN) / (ATOMS - 1)
B = 524288
N_CORES = 8
KT = 130                    # k-tiles (256 rows) per direction group per core
R0 = KT * 256               # 33280 padded rows per group per core
ROWSP = 2 * R0              # 66560 rows per core
ABYTES = 26                 # 51 nibbles packed two-per-byte (one pad nibble)
FB = 52                     # feature bytes per row (51 fp8 bytes + 1 zero pad)
C1 = float(np.log(2.0) / 8.0)

# chunk schedule in k-tiles, per group: small head so compute starts on the
# first arriving bytes, near-uniform middle, small tail to shorten the drain
FCHUNKS = [22, 22, 22, 22, 21, 21]
assert sum(FCHUNKS) == KT
AMAX = max(FCHUNKS)

_NC_CACHE = None
LAST_RESULT = None


def _build_nc():
    import concourse.bass as bass  # noqa: F401
    import concourse.tile as tile
    from concourse import bacc, mybir

    nc = bacc.Bacc(
        "TRN2",
        target_bir_lowering=False,
        debug=False,
        enable_asserts=True,
        num_devices=N_CORES,
        enable_partition_id=False,
    )
    f32 = mybir.dt.float32
    fp8 = mybir.dt.float8e4
    u16 = mybir.dt.uint16
    u8 = mybir.dt.uint8
    AND = mybir.AluOpType.bitwise_and
    LSR = mybir.AluOpType.logical_shift_right
    DR = mybir.MatmulPerfMode.DoubleRow

    aw4 = nc.dram_tensor("aw4", [ROWSP, ABYTES], u8, kind="ExternalInput").ap()
    feat = nc.dram_tensor("feat", [ROWSP, FB], u8, kind="ExternalInput").ap()
    # out: per group g: [52, 104] = [lo(51) junk hi(51) junk] at cols g*104
    acc = nc.dram_tensor("acc", [52, 2 * 2 * FB], f32, kind="ExternalOutput").ap()

    with ExitStack() as ctx:
        tc = ctx.enter_context(tile.TileContext(nc))
        singles = ctx.enter_context(tc.tile_pool(name="singles", bufs=1))
        floads = ctx.enter_context(tc.tile_pool(name="floads", bufs=5))
        aloads = ctx.enter_context(tc.tile_pool(name="aloads", bufs=4))
        planes = ctx.enter_context(tc.tile_pool(name="planes", bufs=4))
        psums = ctx.enter_context(tc.tile_pool(name="psums", bufs=1, space="PSUM"))

        # M=64: DoubleRow needs out partitions in {32,64,128}; rows 52-63 are
        # driven by the 12 pad bytes per weight row, never copied out
        ps = [psums.tile([64, 2 * FB], f32, name=f"ps_{g}") for g in (0, 1)]
        out_sb = singles.tile([52, 2 * 2 * FB], f32)

        # persistent 2-slot anchor-plane buffer; pad lanes zeroed once.
        # layout (slot, k-tile, t, 32 lanes): lanes 0-12 lo plane (even anchor
        # cols), 13-25 hi plane (odd cols), 26-31 pad
        ap_slots = singles.tile([128, 4, AMAX, 2, 32], u16)
        nc.vector.memset(ap_slots[:, :, :, :, 26:32], 0)
        ap8 = ap_slots.bitcast(fp8)        # [128, 2, AMAX, 2, 64]

        # chunk list
        fsched = []                        # (g, rck, kt_base, first, last)
        for g in (0, 1):
            kt = 0
            for rck in FCHUNKS:
                fsched.append((g, rck, kt, kt == 0, kt + rck == KT))
                kt += rck

        f_tiles = [None] * len(fsched)
        a_tiles = [None] * len(fsched)

        def issue_dma(ci):
            g, rck, kt0, _, _ = fsched[ci]
            row0 = g * R0 + kt0 * 256
            q_f = nc.sync if ci % 2 == 0 else nc.scalar
            q_a = nc.scalar if ci % 2 == 0 else nc.sync
            f_t = floads.tile([128, rck * FB], u16, tag="f", name=f"f_{ci}")
            q_f.dma_start(
                out=f_t.bitcast(u8).rearrange("p (r j) -> p r j", j=FB),
                in_=feat[row0 : row0 + 256 * rck, :].rearrange(
                    "(p r) j -> p r j", r=2 * rck
                ),
            )
            a_t = aloads.tile([128, rck * ABYTES], u16, tag="a", name=f"a_{ci}")
            q_a.dma_start(
                out=a_t.bitcast(u8).rearrange("p (r j) -> p r j", j=ABYTES),
                in_=aw4[row0 : row0 + 256 * rck, :].rearrange(
                    "(p r) j -> p r j", r=2 * rck
                ),
            )
            f_tiles[ci] = f_t
            a_tiles[ci] = a_t

        nf = len(fsched)
        for ci0 in range(3):
            issue_dma(ci0)

        for ci, (g, rck, kt0, first, last) in enumerate(fsched):
            if ci + 3 < nf:
                issue_dma(ci + 3)
            f_t, a_t = f_tiles[ci], a_tiles[ci]
            s = ci % 4

            # nibble planes.
            # fpl: (k-tile, t, plane, 26 lanes) -> rhs free merges to 104
            # apl: lanes 0-12 lo plane (even cols), 13-25 hi (odd), 26-31 pad
            fpl = planes.tile([128, rck, 2, 2, FB // 2], u16, tag="fp",
                              name=f"fp_{ci}")
            nc.vector.tensor_scalar(fpl[:, :, :, 0], f_t, 0x0F0F, None, AND)
            nc.vector.tensor_scalar(fpl[:, :, :, 1], f_t, 4, 0x0F0F, LSR, AND)
            nc.vector.tensor_scalar(
                ap_slots[:, s, :rck, :, 0:13], a_t, 0x0F0F, None, AND)
            nc.vector.tensor_scalar(
                ap_slots[:, s, :rck, :, 13:26], a_t, 4, 0x0F0F, LSR, AND)

            fpl8 = fpl.bitcast(fp8)   # [128, rck, 2, 2, 52]
            for q in range(rck):
                # one ldweights + one matmul per 256-row k-tile
                nc.tensor.matmul(
                    ps[g],
                    lhsT=ap8[:, s, q],
                    rhs=fpl8[:, q],
                    start=(first and q == 0),
                    stop=(last and q == rck - 1),
                    perf_mode=DR,
                    skip_group_check=True,
                )

            if last:
                # group 0 drains mid-stream; per-group out DMA on the ACT queue
                nc.vector.tensor_copy(
                    out_sb[:, g * 2 * FB : (g + 1) * 2 * FB], ps[g][0:52]
                )
                nc.scalar.dma_start(
                    out=acc[:, g * 2 * FB : (g + 1) * 2 * FB],
                    in_=out_sb[:, g * 2 * FB : (g + 1) * 2 * FB],
                )

    nc.compile()
    return nc


def _get_nc():
    global _NC_CACHE
    if _NC_CACHE is None:
        _NC_CACHE = _build_nc()
    return _NC_CACHE


def _build_P(skew):
    """51x51 projection matrix for scalar skew, replicating reference f32 ops."""
    supports = np.linspace(V_MIN, V_MAX, ATOMS, dtype=np.float32)
    Tz = np.clip(np.float32(skew) + supports, np.float32(V_MIN), np.float32(V_MAX))
    b = (Tz - np.float32(V_MIN)) / np.float32(DELTA)
    l = np.floor(b).astype(np.int32)
    u = np.ceil(b).astype(np.int32)
    eq = l == u
    l = np.where((u > 0) & eq, l - 1, l)
    u = np.where((l < ATOMS - 1) & (l == u), u + 1, u)
    wl = u.astype(np.float32) - b
    wu = b - l.astype(np.float32)
    P = np.zeros((ATOMS, ATOMS), dtype=np.float64)
    np.add.at(P, (np.arange(ATOMS), l), wl.astype(np.float64))
    np.add.at(P, (np.arange(ATOMS), u), wu.astype(np.float64))
    return P


def encode_host(anchor, feature, direction, weight):
    """Quantize + sort + shard. Returns (in_maps, colq52 [2,52], K [2,51])."""
    import ml_dtypes

    FP8 = ml_dtypes.float8_e4m3
    anchor = np.asarray(anchor, dtype=np.float32)
    feature = np.asarray(feature, dtype=np.float32)
    w = np.asarray(weight, dtype=np.float32)
    mask1 = np.asarray(direction) == 1

    aw = anchor * w[:, None]                                   # [B,51] in [0,1)
    rng = np.random.default_rng(1234)
    q = np.floor(aw * np.float32(15.0)
                 + rng.random(aw.shape, dtype=np.float32)).astype(np.uint8)
    qp = np.zeros((B, 2 * ABYTES), dtype=np.uint8)
    qp[:, :ATOMS] = q
    aw4 = qp[:, 0::2] | (qp[:, 1::2] << 4)                     # [B,26]

    fbits = np.zeros((B, FB), dtype=np.uint8)
    fbits[:, :ATOMS] = np.maximum(
        (feature * np.float32(128.0)).astype(FP8).view(np.uint8), 1
    )                                                          # [B,52], pad col 0

    # host-side correction stats per direction group (integer bookkeeping)
    vtab = np.arange(256, dtype=np.uint8).view(FP8).astype(np.float64)
    valid = np.isfinite(vtab) & (vtab > 0)
    assert valid[int(fbits[:, :ATOMS].max())]
    with np.errstate(invalid="ignore", divide="ignore"):
        r_tab = np.where(valid, np.log(vtab / 128.0) - C1 * np.arange(256), 0.0)

    even = np.arange(0, ATOMS, 2)
    odd = np.arange(1, ATOMS, 2)
    colq52 = np.zeros((2, 52))
    K = np.zeros((2, ATOMS))
    coloff = (np.arange(ATOMS) << 8).astype(np.int32)
    for g, m in enumerate([~mask1, mask1]):
        cq = q[m].astype(np.float64).sum(axis=0)               # [51] nibble units
        colq52[g, :26] = cq[even]
        colq52[g, 26:51] = cq[odd]
        h = np.bincount((fbits[m, :ATOMS].astype(np.int32) + coloff).ravel(),
                        minlength=256 * ATOMS).reshape(ATOMS, 256)
        G = int(m.sum())
        K[g] = (h * r_tab).sum(axis=1) / G

    # stable sort by direction, shard each group across cores, zero-pad
    order = np.argsort(mask1, kind="stable")
    n0 = int((~mask1).sum())
    aw4_s = aw4[order]
    fbits_s = fbits[order]
    assert n0 <= N_CORES * R0 and (B - n0) <= N_CORES * R0, "group overflow"

    in_maps = []
    b0 = np.linspace(0, n0, N_CORES + 1).astype(int)
    b1 = np.linspace(n0, B, N_CORES + 1).astype(int)
    for c in range(N_CORES):
        a_c = np.zeros((ROWSP, ABYTES), dtype=np.uint8)
        f_c = np.zeros((ROWSP, FB), dtype=np.uint8)
        s0, e0 = b0[c], b0[c + 1]
        s1, e1 = b1[c], b1[c + 1]
        a_c[: e0 - s0] = aw4_s[s0:e0]
        f_c[: e0 - s0] = fbits_s[s0:e0]
        a_c[R0 : R0 + e1 - s1] = aw4_s[s1:e1]
        f_c[R0 : R0 + e1 - s1] = fbits_s[s1:e1]
        in_maps.append({"aw4": a_c, "feat": f_c})
    return in_maps, colq52, K


def reduce_host(accs, skewness, colq52, K):
    """accs: list of per-core [52, 208] f32."""
    acc = np.zeros((52, 2 * 2 * FB), dtype=np.float64)
    for a in accs:
        acc += np.asarray(a, dtype=np.float64)
    even = np.arange(0, ATOMS, 2)
    odd = np.arange(1, ATOMS, 2)
    P_mats = [_build_P(np.float32(skewness)), _build_P(np.float32(-np.float32(skewness)))]
    contrib = 0.0
    for g in (0, 1):
        lo = acc[:, g * 2 * FB : g * 2 * FB + ATOMS]
        hi = acc[:, g * 2 * FB + FB : g * 2 * FB + FB + ATOMS]
        SX = (16.0 * hi + lo) * float(2.0**18)       # sum_i qa[i,m] * byte[i,n]
        M52 = (C1 * SX + np.outer(colq52[g], K[g])) / 15.0
        M = np.zeros((ATOMS, ATOMS))
        M[even] = M52[:26]
        M[odd] = M52[26:51]
        contrib += (P_mats[g] * M).sum()
    return np.asarray(np.float32(-contrib / B))


def run_device(in_maps, trace=False):
    global LAST_RESULT
    from concourse.bass_utils import run_bass_kernel_spmd

    LAST_RESULT = run_bass_kernel_spmd(
        _get_nc(), in_maps, core_ids=list(range(N_CORES)), trace=trace
    )
    return LAST_RESULT.results


def kernel(anchor, feature, skewness, direction, weight):
    in_maps, colq52, K = encode_host(anchor, feature, direction, weight)
    results = run_device(in_maps, trace=bool(os.environ.get("KERNEL_TRACE")))
    return reduce_host([r["acc"] for r in results], skewness, colq52, K)
